# revision 11
# baseline (speedup 1.0000x reference)
"""KMaxPooling (top-8 along seq axis) Bass kernel for TRN2, 8-core SPMD.

Input  x: (64, 4096, 256) fp32. Output: (64, 8, 256) fp32 = per (batch,
channel) the 8 largest values over the 4096 seq positions, descending.

Strategy (per core, batch-sharded 8 ways -> 8 batches/core, 32 MB):
  - contiguous loads: partition p holds rows r0 + p*T .. r0 + p*T + T-1
    of a row range, so every partition line is one big (4-16 KB) DMA
    descriptor and the whole load reads HBM sequentially. Top-8 over seq
    is permutation invariant, so the block shuffle is harmless.
  - ALL loads ride ONE HWDGE queue: the 16 DMA engines are a shared
    pool, so concurrent queues only delay first-completion. FIFO on one
    queue keeps completions in issue order at full (~470 GB/s) rate.
  - load sizes: first batch in 1 MB quarters (prime the pipeline fast),
    middle batches in 2 MB halves, last batch 2M/1M/.5M/.5M (short tail).
  - PE transposes 128x128 blocks into PSUM so channels land on partitions
  - DVE InstMax (hardware top-8, sorted desc) over PSUM spans; DVE is the
    critical engine (~1 elem/cycle @ 0.96 GHz -> ~68 us/core minimum)
  - tiny second-level InstMax merges the per-span candidates
  - one 64 KB output DMA per core; host reassembles pure layout
"""

import sys

sys.path.insert(0, "/opt/trn_rl_repo")

import numpy as np

import concourse.bass as bass
import concourse.mybir as mybir
from concourse import masks
from concourse.tile import TileContext
from concourse.vector_clock import ScopedClock, VectorClock
from concourse.bass_utils import run_bass_kernel_spmd

B, S, C, K = 64, 4096, 256, 8
NCORES = 8
BPC = B // NCORES  # batches per core
CH_GROUPS = C // 128  # 2

F32 = mybir.dt.float32
F32R = mybir.dt.float32r  # 4-byte transpose runs 1.5 cycles/row vs 2.0 for f32

N_PROCS = 27


class SplitDrainTileContext(TileContext):
    """The walrus backend here rejects any instruction carrying more than
    one sync wait ("Too many sync wait commands"), but Tile's semaphore
    assignment can attach several. Two fixes:

    1. _lower_ordered_insts: before lowering, hoist excess waits of every
       scheduled instruction onto single-wait same-engine NoOps inserted
       right before it.
    2. _drain_and_barrier: emit one single-wait drain per logical proc
       instead of one drain waiting on the whole global vector clock.
    """

    def _lower_ordered_insts(self, ordered):
        for bb_name, insts in ordered.items():
            rewritten = []
            for inst in insts:
                si = inst.sync_info
                if si is not None and si.on_wait and len(si.on_wait) > 1:
                    waits = list(si.on_wait)
                    for k, w in enumerate(waits[:-1]):
                        nop = mybir.InstNoOp(
                            name=f"{inst.name}.wsplit{k}",
                            engine=inst.engine,
                            sync_info=mybir.SyncInfo(on_wait=[w], on_update=[]),
                            bass_nofuse=True,
                        )
                        rewritten.append(nop)
                    si.on_wait = waits[-1:]
                rewritten.append(inst)
            ordered[bb_name] = rewritten
        return super()._lower_ordered_insts(ordered)

    def _drain_and_barrier(self, tick_clock, wait_clock):
        gc = tick_clock.global_clock
        for p in range(N_PROCS):
            if gc[p] > 0:
                v = [0] * N_PROCS
                v[p] = gc[p]
                di = self.nc.sync.drain()
                wait_clock.add_sem_waits(di.ins, ScopedClock({None: VectorClock(v)}))

        self.nc.all_engine_barrier()
        assert self.sems is not None
        popped = self.nc._tile_sem_poison_stack.pop()
        assert popped is self._sem_poison
        self.nc.clear_and_free_semaphores(list(self.sems.allocated().values()))
        self.nc.all_engine_barrier()


def build_program():
    nc = bass.Bass()
    x_ext = nc.declare_dram_parameter("x", [BPC, S, C], F32, isOutput=False)
    # out[c', g*64 + b*8 + k]: top-k values of channel g*128+c' in batch b
    out_ext = nc.declare_dram_parameter(
        "out", [128, CH_GROUPS * BPC * K], F32, isOutput=True
    )

    with SplitDrainTileContext(nc) as tc:
        with (
            tc.tile_pool(name="const", bufs=1) as const_pool,
            tc.tile_pool(name="xin", bufs=8) as in_pool,
            tc.tile_pool(name="psum", bufs=4, space="PSUM") as psum_pool,
            tc.tile_pool(name="cand", bufs=4) as cand_pool,
            tc.tile_pool(name="obuf", bufs=1) as out_pool,
        ):
            identity = const_pool.tile([128, 128], F32)
            masks.make_identity(nc, identity[:])

            obuf = out_pool.tile([128, CH_GROUPS * BPC * K], F32)

            def load_rows(b, r0, r1):
                """One contiguous DMA of x[b, r0:r1] with partition p
                holding rows r0 + p*T .. r0 + p*T + T-1 (T KB descriptor
                per partition). All loads ride the sync-engine queue so
                completions arrive FIFO. Returns (xin, T)."""
                nrows = r1 - r0
                T = nrows // 128
                xin = in_pool.tile([128, T * C], F32, name="xin", tag="xin")
                nc.sync.dma_start(
                    out=xin[:],
                    in_=x_ext[b, r0:r1].rearrange("(p t) c -> p (t c)", p=128),
                )
                return xin, T

            def transpose_span(xin, g, ps, s0, t0, nt):
                """Transpose blocks t0..t0+nt-1 of group g from xin into
                ps columns starting at slot s0 (128 cols per slot)."""
                for i in range(nt):
                    col = (t0 + i) * C + g * 128
                    s = s0 + i
                    nc.tensor.matmul(
                        ps[:, 128 * s : 128 * (s + 1)],
                        xin[:, col : col + 128],
                        identity[:],
                        is_transpose=True,
                        start=True,
                        stop=True,
                    )

            def process_spans(xin, spans, cands):
                """Transpose `xin` into fresh 1024-wide PSUM tiles (2 banks
                each, 4 bufs -> 2-3 spans of elasticity between PE and DVE)
                and top-8 each tile into cands[g][:, slot*K:...]. `spans` is
                the number of 128-col blocks per group in this load; it is
                split into ceil(spans/8) PSUM tiles per group."""
                nonlocal slot_cursor
                assert spans >= 4
                for t0 in range(0, spans, 8):
                    nt = min(8, spans - t0)
                    for g in range(CH_GROUPS):
                        ps = psum_pool.tile([128, 1024], F32, name="ps", tag="ps")
                        transpose_span(xin, g, ps, 0, t0, nt)
                        nc.vector.max(
                            out=cands[g][:, slot_cursor * K : (slot_cursor + 1) * K],
                            in_=ps[:, : 128 * nt],
                        )
                    slot_cursor += 1

            def merge(cands, b):
                for g in range(CH_GROUPS):
                    nc.vector.max(
                        out=obuf[:, (g * BPC + b) * K : (g * BPC + b + 1) * K],
                        in_=cands[g][:],
                    )

            # Per-batch load plans (row cuts). Batch 0 ramps up from 512 KB
            # so the first MAX8 fires early; the last batch ramps down so the
            # tail after the final DMA packet is short; middle batches use
            # 2 MB halves.
            ramp = [0, S // 8, S // 4, S // 2, S]
            half = [0, S // 2, S]
            plans = (
                [ramp] + [half] * (BPC - 2) + [[0, S // 2, 3 * S // 4, 7 * S // 8, S]]
            )
            for b, cuts in enumerate(plans):
                nslots = sum(
                    -(-((cuts[i + 1] - cuts[i]) // 128) // 8)
                    for i in range(len(cuts) - 1)
                )
                cands = [
                    cand_pool.tile([128, nslots * K], F32, name="cand", tag="cand")
                    for _ in range(CH_GROUPS)
                ]
                slot_cursor = 0
                for i in range(len(cuts) - 1):
                    xin, T = load_rows(b, cuts[i], cuts[i + 1])
                    process_spans(xin, T, cands)
                merge(cands, b)

            nc.sync.dma_start(out=out_ext[:], in_=obuf[:])

    return nc


_prog = None


def _get_prog():
    global _prog
    if _prog is None:
        _prog = build_program()
    return _prog


def run_on_cores(x: np.ndarray, **run_kwargs):
    """Shard, run on 8 cores, return (full_output, BassKernelResults)."""
    nc = _get_prog()
    x = np.ascontiguousarray(np.asarray(x, dtype=np.float32))
    in_maps = [
        {"x": np.ascontiguousarray(x[i * BPC : (i + 1) * BPC])} for i in range(NCORES)
    ]
    res = run_bass_kernel_spmd(nc, in_maps, list(range(NCORES)), **run_kwargs)
    parts = []
    for i in range(NCORES):
        o = res.results[i]["out"]  # (128, CH_GROUPS*BPC*K)
        o = o.reshape(128, CH_GROUPS, BPC, K)  # (c', g, b, k)
        o = o.transpose(2, 3, 1, 0).reshape(BPC, K, C)  # (b, k, g*128+c')
        parts.append(o)
    return np.concatenate(parts, axis=0), res


def kernel(x: np.ndarray) -> np.ndarray:
    out, _ = run_on_cores(x)
    return out


# revision 12
# speedup vs baseline: 1.0816x; 1.0816x over previous
"""KMaxPooling (top-8 along seq axis) Bass kernel for TRN2, 8-core SPMD.

Input  x: (64, 4096, 256) fp32. Output: (64, 8, 256) fp32 = per (batch,
channel) the 8 largest values over the 4096 seq positions, descending.

Strategy (per core, batch-sharded 8 ways -> 8 batches/core, 32 MB):
  - casting loads: gpsimd (SWDGE) DMAs read fp32 HBM and write bf16 SBUF
    (only the Pool engine may issue casting DMAs). bf16 keeps rel err
    <= 2e-3, well under the 2e-2 gate, and it halves SBUF footprint,
    halves PE transpose time (1 cycle/row vs 2 for fp32 — also half the
    PE energy, which matters because the package power-throttles), and
    halves PSUM footprint so a whole batch-group fits in one PSUM tile.
  - contiguous loads: partition p holds rows r0+p*T .. r0+p*T+T-1, so
    every partition line is one 4-16 KB descriptor and the whole load
    reads HBM sequentially. Top-8 over seq is permutation invariant, so
    the resulting block shuffle is harmless. All loads ride ONE queue:
    the 16 DMA engines are a shared pool, so concurrent queues only
    delay first-completion; FIFO keeps completions in issue order.
  - load sizes ramp up 512K/512K/1M/2M for batch 0 (first span ready
    early), 2 MB halves for middle batches, and ramp down
    2M/1M/512K/512K for the last batch (short tail).
  - PE transposes 128x128 bf16 blocks into PSUM; a full (batch, group)
    span (4096 values) is one 8 KB PSUM tile (4 banks), so DVE does ONE
    InstMax per (batch, group) straight into the output buffer — no
    second-level merges. DVE is the critical engine (InstMax is ~1
    elem/cycle @ 0.96 GHz regardless of dtype -> ~67 us/core).
  - the last batch keeps per-load spans + a tiny merge so the tail after
    the final DMA packet stays ~2 us.
  - one upcast copy (Act engine) + one 64 KB output DMA per core; host
    reassembles pure layout.
"""

import sys

sys.path.insert(0, "/opt/trn_rl_repo")

import numpy as np

import concourse.bass as bass
import concourse.mybir as mybir
from concourse import masks
from concourse.tile import TileContext
from concourse.vector_clock import ScopedClock, VectorClock
from concourse.bass_utils import run_bass_kernel_spmd

B, S, C, K = 64, 4096, 256, 8
NCORES = 8
BPC = B // NCORES  # batches per core
CH_GROUPS = C // 128  # 2

F32 = mybir.dt.float32
BF16 = mybir.dt.bfloat16

N_PROCS = 27


class SplitDrainTileContext(TileContext):
    """The walrus backend here rejects any instruction carrying more than
    one sync wait ("Too many sync wait commands"), but Tile's semaphore
    assignment can attach several. Two fixes:

    1. _lower_ordered_insts: before lowering, hoist excess waits of every
       scheduled instruction onto single-wait same-engine NoOps inserted
       right before it.
    2. _drain_and_barrier: emit one single-wait drain per logical proc
       instead of one drain waiting on the whole global vector clock.
    """

    def _lower_ordered_insts(self, ordered):
        for bb_name, insts in ordered.items():
            rewritten = []
            for inst in insts:
                si = inst.sync_info
                if si is not None and si.on_wait and len(si.on_wait) > 1:
                    waits = list(si.on_wait)
                    for k, w in enumerate(waits[:-1]):
                        nop = mybir.InstNoOp(
                            name=f"{inst.name}.wsplit{k}",
                            engine=inst.engine,
                            sync_info=mybir.SyncInfo(on_wait=[w], on_update=[]),
                            bass_nofuse=True,
                        )
                        rewritten.append(nop)
                    si.on_wait = waits[-1:]
                rewritten.append(inst)
            ordered[bb_name] = rewritten
        return super()._lower_ordered_insts(ordered)

    def _drain_and_barrier(self, tick_clock, wait_clock):
        gc = tick_clock.global_clock
        for p in range(N_PROCS):
            if gc[p] > 0:
                v = [0] * N_PROCS
                v[p] = gc[p]
                di = self.nc.sync.drain()
                wait_clock.add_sem_waits(di.ins, ScopedClock({None: VectorClock(v)}))

        self.nc.all_engine_barrier()
        assert self.sems is not None
        popped = self.nc._tile_sem_poison_stack.pop()
        assert popped is self._sem_poison
        self.nc.clear_and_free_semaphores(list(self.sems.allocated().values()))
        self.nc.all_engine_barrier()


def build_program():
    nc = bass.Bass()
    x_ext = nc.declare_dram_parameter("x", [BPC, S, C], F32, isOutput=False)
    # out[c', g*64 + b*8 + k]: top-k values of channel g*128+c' in batch b
    out_ext = nc.declare_dram_parameter(
        "out", [128, CH_GROUPS * BPC * K], F32, isOutput=True
    )

    with SplitDrainTileContext(nc) as tc:
        with (
            tc.tile_pool(name="const", bufs=1) as const_pool,
            tc.tile_pool(name="xin", bufs=8) as in_pool,
            tc.tile_pool(name="psum", bufs=2, space="PSUM") as psum_pool,
            tc.tile_pool(name="cand", bufs=2) as cand_pool,
            tc.tile_pool(name="obuf", bufs=1) as out_pool,
        ):
            identity = const_pool.tile([128, 128], BF16)
            masks.make_identity(nc, identity[:])

            obuf = out_pool.tile([128, CH_GROUPS * BPC * K], BF16)
            obuf_f32 = out_pool.tile([128, CH_GROUPS * BPC * K], F32)

            def load_rows(b, r0, r1):
                """One contiguous casting DMA of x[b, r0:r1] (fp32 HBM ->
                bf16 SBUF) with partition p holding rows r0+p*T..r0+p*T+T-1.
                All loads ride the gpsimd SWDGE queue so completions arrive
                FIFO. Returns (xin, T)."""
                nrows = r1 - r0
                T = nrows // 128
                xin = in_pool.tile([128, T * C], BF16, name="xin", tag="xin")
                nc.gpsimd.dma_start(
                    out=xin[:],
                    in_=x_ext[b, r0:r1].rearrange("(p t) c -> p (t c)", p=128),
                )
                return xin, T

            def transpose_blocks(xin, T, g, ps, s0):
                """Transpose group g's T blocks of `xin` into ps at slot s0."""
                for i in range(T):
                    col = i * C + g * 128
                    s = s0 + i
                    nc.tensor.matmul(
                        ps[:, 128 * s : 128 * (s + 1)],
                        xin[:, col : col + 128],
                        identity[:],
                        is_transpose=True,
                        start=True,
                        stop=True,
                    )

            def oslot(b, g):
                return slice((g * BPC + b) * K, (g * BPC + b + 1) * K)

            # batches 0..BPC-2: whole-batch PSUM spans, ONE MAX8 per
            # (batch, group), no merges. Batch 0's loads ramp up so the
            # pipeline primes quickly.
            ramp = [0, S // 8, S // 4, S // 2, S]
            half = [0, S // 2, S]
            for b in range(BPC - 1):
                cuts = ramp if b == 0 else half
                pss = [
                    psum_pool.tile([128, S], BF16, name="ps", tag="ps")
                    for _ in range(CH_GROUPS)
                ]
                for i in range(len(cuts) - 1):
                    xin, T = load_rows(b, cuts[i], cuts[i + 1])
                    for g in range(CH_GROUPS):
                        transpose_blocks(xin, T, g, pss[g], cuts[i] // 128)
                for g in range(CH_GROUPS):
                    nc.vector.max(out=obuf[:, oslot(b, g)], in_=pss[g][:])

            # last batch: per-load spans + a small merge so the tail after
            # the final DMA packet stays short. Spans live in separate PSUM
            # tiles (uniform 4-bank slots, small spans use a prefix).
            b = BPC - 1
            cuts = [0, S // 2, 3 * S // 4, 7 * S // 8, S]
            cands = [
                cand_pool.tile([128, 4 * K], BF16, name="cand", tag="cand")
                for _ in range(CH_GROUPS)
            ]
            for i in range(len(cuts) - 1):
                xin, T = load_rows(b, cuts[i], cuts[i + 1])
                for g in range(CH_GROUPS):
                    ps = psum_pool.tile([128, S], BF16, name="ps", tag="ps")
                    transpose_blocks(xin, T, g, ps, 0)
                    nc.vector.max(
                        out=cands[g][:, i * K : (i + 1) * K],
                        in_=ps[:, : 128 * T],
                    )
            for g in range(CH_GROUPS):
                nc.vector.max(out=obuf[:, oslot(b, g)], in_=cands[g][:])

            nc.scalar.copy(out=obuf_f32[:], in_=obuf[:])
            nc.sync.dma_start(out=out_ext[:], in_=obuf_f32[:])

    return nc


_prog = None


def _get_prog():
    global _prog
    if _prog is None:
        _prog = build_program()
    return _prog


def run_on_cores(x: np.ndarray, **run_kwargs):
    """Shard, run on 8 cores, return (full_output, BassKernelResults)."""
    nc = _get_prog()
    x = np.ascontiguousarray(np.asarray(x, dtype=np.float32))
    in_maps = [
        {"x": np.ascontiguousarray(x[i * BPC : (i + 1) * BPC])} for i in range(NCORES)
    ]
    res = run_bass_kernel_spmd(nc, in_maps, list(range(NCORES)), **run_kwargs)
    parts = []
    for i in range(NCORES):
        o = res.results[i]["out"]  # (128, CH_GROUPS*BPC*K)
        o = o.reshape(128, CH_GROUPS, BPC, K)  # (c', g, b, k)
        o = o.transpose(2, 3, 1, 0).reshape(BPC, K, C)  # (b, k, g*128+c')
        parts.append(o)
    return np.concatenate(parts, axis=0), res


def kernel(x: np.ndarray) -> np.ndarray:
    out, _ = run_on_cores(x)
    return out


# revision 14
# speedup vs baseline: 1.0922x; 1.0098x over previous
"""KMaxPooling (top-8 along seq axis) Bass kernel for TRN2, 8-core SPMD.

Input  x: (64, 4096, 256) fp32. Output: (64, 8, 256) fp32 = per (batch,
channel) the 8 largest values over the 4096 seq positions, descending.

Strategy (per core, batch-sharded 8 ways -> 8 batches/core, 32 MB):
  - casting loads: gpsimd (SWDGE) DMAs read fp32 HBM and write bf16 SBUF
    (only the Pool engine may issue casting DMAs). bf16 keeps rel err
    <= 2e-3, well under the 2e-2 gate, and it halves SBUF footprint,
    halves PE transpose time (1 cycle/row vs 2 for fp32 — also half the
    PE energy, which matters because the package power-throttles), and
    halves PSUM footprint so a whole batch-group fits in one PSUM tile.
  - contiguous loads: partition p holds rows r0+p*T .. r0+p*T+T-1, so
    every partition line is one 4-16 KB descriptor and the whole load
    reads HBM sequentially. Top-8 over seq is permutation invariant, so
    the resulting block shuffle is harmless. All loads ride ONE queue:
    the 16 DMA engines are a shared pool, so concurrent queues only
    delay first-completion; FIFO keeps completions in issue order.
  - load sizes ramp up 512K/512K/1M/2M for batch 0 (first span ready
    early), 2 MB halves for middle batches, and ramp down
    2M/1M/512K/512K for the last batch (short tail).
  - PE transposes 128x128 bf16 blocks into PSUM; a full (batch, group)
    span (4096 values) is one 8 KB PSUM tile (4 banks), so DVE does ONE
    InstMax per (batch, group) straight into the output buffer — no
    second-level merges. DVE is the critical engine (InstMax is ~1
    elem/cycle @ 0.96 GHz regardless of dtype -> ~67 us/core).
  - the last batch keeps per-load spans + a tiny merge so the tail after
    the final DMA packet stays ~2 us.
  - one upcast copy (Act engine) + one 64 KB output DMA per core; host
    reassembles pure layout.
"""

import sys

sys.path.insert(0, "/opt/trn_rl_repo")

import numpy as np

import concourse.bass as bass
import concourse.mybir as mybir
from concourse import masks
from concourse.tile import TileContext
from concourse.vector_clock import ScopedClock, VectorClock
from concourse.bass_utils import run_bass_kernel_spmd

B, S, C, K = 64, 4096, 256, 8
NCORES = 8
BPC = B // NCORES  # batches per core
CH_GROUPS = C // 128  # 2

F32 = mybir.dt.float32
BF16 = mybir.dt.bfloat16

N_PROCS = 27


class SplitDrainTileContext(TileContext):
    """The walrus backend here rejects any instruction carrying more than
    one sync wait ("Too many sync wait commands"), but Tile's semaphore
    assignment can attach several. Two fixes:

    1. _lower_ordered_insts: before lowering, hoist excess waits of every
       scheduled instruction onto single-wait same-engine NoOps inserted
       right before it.
    2. _drain_and_barrier: emit one single-wait drain per logical proc
       instead of one drain waiting on the whole global vector clock.
    """

    def _lower_ordered_insts(self, ordered):
        for bb_name, insts in ordered.items():
            rewritten = []
            for inst in insts:
                si = inst.sync_info
                if si is not None and si.on_wait and len(si.on_wait) > 1:
                    waits = list(si.on_wait)
                    for k, w in enumerate(waits[:-1]):
                        nop = mybir.InstNoOp(
                            name=f"{inst.name}.wsplit{k}",
                            engine=inst.engine,
                            sync_info=mybir.SyncInfo(on_wait=[w], on_update=[]),
                            bass_nofuse=True,
                        )
                        rewritten.append(nop)
                    si.on_wait = waits[-1:]
                rewritten.append(inst)
            ordered[bb_name] = rewritten
        return super()._lower_ordered_insts(ordered)

    def _drain_and_barrier(self, tick_clock, wait_clock):
        gc = tick_clock.global_clock
        for p in range(N_PROCS):
            if gc[p] > 0:
                v = [0] * N_PROCS
                v[p] = gc[p]
                di = self.nc.sync.drain()
                wait_clock.add_sem_waits(di.ins, ScopedClock({None: VectorClock(v)}))

        self.nc.all_engine_barrier()
        assert self.sems is not None
        popped = self.nc._tile_sem_poison_stack.pop()
        assert popped is self._sem_poison
        self.nc.clear_and_free_semaphores(list(self.sems.allocated().values()))
        self.nc.all_engine_barrier()


def build_program():
    nc = bass.Bass()
    x_ext = nc.declare_dram_parameter("x", [BPC, S, C], F32, isOutput=False)
    # out[c', g*64 + b*8 + k]: top-k values of channel g*128+c' in batch b
    out_ext = nc.declare_dram_parameter(
        "out", [128, CH_GROUPS * BPC * K], F32, isOutput=True
    )

    with SplitDrainTileContext(nc) as tc:
        with (
            tc.tile_pool(name="const", bufs=1) as const_pool,
            tc.tile_pool(name="xin", bufs=8) as in_pool,
            tc.tile_pool(name="psum", bufs=2, space="PSUM") as psum_pool,
            tc.tile_pool(name="cand", bufs=2) as cand_pool,
            tc.tile_pool(name="obuf", bufs=1) as out_pool,
        ):
            identity = const_pool.tile([128, 128], BF16)
            masks.make_identity(nc, identity[:])

            obuf = out_pool.tile([128, CH_GROUPS * BPC * K], BF16)
            obuf_f32 = out_pool.tile([128, CH_GROUPS * BPC * K], F32)

            def load_rows(b, r0, r1):
                """One contiguous casting DMA of x[b, r0:r1] (fp32 HBM ->
                bf16 SBUF) with partition p holding rows r0+p*T..r0+p*T+T-1.
                All loads ride the gpsimd SWDGE queue so completions arrive
                FIFO. Returns (xin, T)."""
                nrows = r1 - r0
                T = nrows // 128
                xin = in_pool.tile([128, T * C], BF16, name="xin", tag="xin")
                nc.gpsimd.dma_start(
                    out=xin[:],
                    in_=x_ext[b, r0:r1].rearrange("(p t) c -> p (t c)", p=128),
                )
                return xin, T

            def transpose_blocks(xin, T, g, ps, s0):
                """Transpose group g's T blocks of `xin` into ps at slot s0."""
                for i in range(T):
                    col = i * C + g * 128
                    s = s0 + i
                    nc.tensor.matmul(
                        ps[:, 128 * s : 128 * (s + 1)],
                        xin[:, col : col + 128],
                        identity[:],
                        is_transpose=True,
                        start=True,
                        stop=True,
                    )

            def oslot(b, g):
                return slice((g * BPC + b) * K, (g * BPC + b + 1) * K)

            def per_load_spans(b, cuts):
                """Per-load PSUM spans + a small candidate merge: the first
                MAX8 fires as soon as the first (small) load is transposed.
                Used for the first batch (prime the DVE pipeline early) and
                the last (short tail after the final DMA packet)."""
                cands = [
                    cand_pool.tile(
                        [128, (len(cuts) - 1) * K], BF16, name="cand", tag="cand"
                    )
                    for _ in range(CH_GROUPS)
                ]
                for i in range(len(cuts) - 1):
                    xin, T = load_rows(b, cuts[i], cuts[i + 1])
                    for g in range(CH_GROUPS):
                        ps = psum_pool.tile([128, S], BF16, name="ps", tag="ps")
                        transpose_blocks(xin, T, g, ps, 0)
                        nc.vector.max(
                            out=cands[g][:, i * K : (i + 1) * K],
                            in_=ps[:, : 128 * T],
                        )
                for g in range(CH_GROUPS):
                    nc.vector.max(out=obuf[:, oslot(b, g)], in_=cands[g][:])

            # batch 0: ramped loads with per-load spans -> first MAX8 ~12us
            per_load_spans(0, [0, S // 8, S // 4, S // 2, S])

            # batches 1..BPC-2: 2 MB half loads into a whole-batch PSUM
            # span, ONE MAX8 per (batch, group), no merges. Half loads let
            # PE start transposing while the second half streams in.
            for b in range(1, BPC - 1):
                pss = [
                    psum_pool.tile([128, S], BF16, name="ps", tag="ps")
                    for _ in range(CH_GROUPS)
                ]
                for h in range(2):
                    xin, T = load_rows(b, h * S // 2, (h + 1) * S // 2)
                    for g in range(CH_GROUPS):
                        transpose_blocks(xin, T, g, pss[g], h * 16)
                for g in range(CH_GROUPS):
                    nc.vector.max(out=obuf[:, oslot(b, g)], in_=pss[g][:])

            # last batch: ramp-down loads with per-load spans -> short tail
            per_load_spans(BPC - 1, [0, S // 2, 3 * S // 4, 7 * S // 8, S])

            nc.scalar.copy(out=obuf_f32[:], in_=obuf[:])
            nc.sync.dma_start(out=out_ext[:], in_=obuf_f32[:])

    return nc


_prog = None


def _get_prog():
    global _prog
    if _prog is None:
        _prog = build_program()
    return _prog


def run_on_cores(x: np.ndarray, **run_kwargs):
    """Shard, run on 8 cores, return (full_output, BassKernelResults)."""
    nc = _get_prog()
    x = np.ascontiguousarray(np.asarray(x, dtype=np.float32))
    in_maps = [
        {"x": np.ascontiguousarray(x[i * BPC : (i + 1) * BPC])} for i in range(NCORES)
    ]
    res = run_bass_kernel_spmd(nc, in_maps, list(range(NCORES)), **run_kwargs)
    parts = []
    for i in range(NCORES):
        o = res.results[i]["out"]  # (128, CH_GROUPS*BPC*K)
        o = o.reshape(128, CH_GROUPS, BPC, K)  # (c', g, b, k)
        o = o.transpose(2, 3, 1, 0).reshape(BPC, K, C)  # (b, k, g*128+c')
        parts.append(o)
    return np.concatenate(parts, axis=0), res


def kernel(x: np.ndarray) -> np.ndarray:
    out, _ = run_on_cores(x)
    return out
